# revision 46
# baseline (speedup 1.0000x reference)
"""Block-sparse attention kernel for Trainium2 (8 NeuronCores).

Problem: B=2, S=2048, H=16, Dqk=Dv=64, 64x64 block mask (30% + forced diag),
AND causal. out = softmax(mask(QK^T/8)) @ V.

Strategy
--------
- Shard the 32 (batch, head) pairs across 8 cores, 4 heads per core,
  balanced by per-head schedule cost.
- Each core gets its OWN Bass program with the sparse block schedule baked in
  from its heads' block masks (compiled at call time, run concurrently on the
  8 axon devices).
- Per head, scores are computed TRANSPOSED (S^T[k, q]) so that P^T = exp(S^T)
  lands in SBUF in exactly the layout PV needs (k on partitions) — no on-chip
  transposes anywhere:
    * host supplies Q^T and K^T as [64(d), 2048(s)] fp16, V as [128, 16*65]
      fp16 "v-pair" tiles [V[kb1]; V[kb2]] with a ones column (col 64); V and
      the ones column are pre-scaled by 1/16 so the fp16 outputs can't
      overflow (the host ratio o_un/l is scale-invariant).
    * k-blocks are paired by greedy max-overlap matching of their active-q
      sets, so paired columns are dense (fewer dead halves).
    * per score chunk (8 columns = 1 PSUM bank): one 1-col start=True matmul
      zero-opens the bank, QK runs (lhsT=K^T pair [64,128], rhs=Q^T qb-run)
      accumulate, then CORRECTION matmuls add -2000 onto masked regions
      (dead 64x64 halves and strict-upper triangles of diagonal blocks) so
      exp maps them to exact zeros. All masking lives on the PE — the
      DVE/GPSIMD never touch the score path.
    * exp: one ACT op per chunk (scale=1/8 fused), fp16 out.
    * PV: matmul(lhsT=[V|1/16] pair [128,65], rhs=P^T run) accumulating
      O^T[65, 2048] in PSUM across k-pairs (banks zero-opened up front).
    * O^T (unnormalized, with row 64 = softmax denominator l/16) is copied to
      SBUF fp16 in two pipelined pieces on the DVE (the ACT engine stays
      dedicated to exp) and DMA'd out; the host divides and transposes back.
- Corrections use constant stationaries: for each (top, bottom) mask case a
  [64,128] pattern P_case with P_case.T @ I64 = the -2000 mask; the moving
  operand is a repeated-identity tile, so one matmul corrects a whole run of
  equal-case columns.
- The chunk chain is PE(open+QK+corr) -> ACT(exp) -> PE(PV): PV emission is
  software-pipelined DEPTH chunks behind QK — across head boundaries — so
  the in-order PE queue never stalls on exp. A zeros warm-up stream during
  the input-DMA wait promotes the HAM clock gate to 8/8 (2.4 GHz) before
  compute starts; micro-idle-heavy schedules sit at 1.2 GHz.
- A chunk is 16 columns = 2 PSUM banks; bank A's QK/corr matmuls contract on
  PE array rows 0:63 and bank B's on rows 64:127 (tile row-groups, operands
  duplicated on-chip onto partitions 64:128), emissions interleaved so the
  two banks execute CONCURRENTLY on the half-used (K=64) array.
- Schedule is pair-major (t-major): all columns of one k-pair are emitted
  together so consecutive matmuls share stationary weights.
- Inputs stream per head through double-buffered pools: head s+2's DMA can
  only start once head s is consumed, so the fabric gives early heads full
  bandwidth and compute starts ~2 heads of DMA earlier.
- Softmax uses no running max: inputs are N(0,1) so scores/8 stay in a range
  where exp() is safely finite (exp(~7) ~ 1e3).
"""

import os
import threading
from contextlib import ExitStack

import numpy as np

import concourse.bass as bass
import concourse.tile as tile
from concourse import mybir
from concourse.bass_utils import run_bass_kernel_spmd
from concourse.vector_clock import ScopedClock

# ---------------------------------------------------------------------------
# Tunables (env-overridable for A/B testing)
# ---------------------------------------------------------------------------
PAIRING = os.environ.get("BSA_PAIRING", "match")  # match | adjacent
GAP = int(os.environ.get("BSA_GAP", "0"))
OUT_F16 = os.environ.get("BSA_OUT_F16", "1") == "1"
SPLIT_COPY = os.environ.get("BSA_SPLIT_COPY", "1") == "1"
WARMUP = int(os.environ.get("BSA_WARMUP", "52"))  # 128-row dummy matmuls: ends
# right as head 0's inputs land, having held the PE busy >3.4us to promote
# the HAM clock gate before real compute starts
DEPTH = int(os.environ.get("BSA_DEPTH", "4"))  # PV pipeline depth (chunks)
SKIP_CLEAR = os.environ.get("BSA_SKIP_CLEAR", "0") == "1"
NEG = -2000.0  # score offset for masked regions: exp((s-2000)/8) == 0

# ----------------------------------------------------------------------------
# Workaround: the installed walrus rejects instructions with more than one
# sync wait. Tile's kernel-tail drain attaches every outstanding clock sem to
# one Drain instruction; split them one wait per Drain.
# ----------------------------------------------------------------------------


def _split_drain_and_barrier(self, tick_clock, wait_clock):
    nc = self.nc
    drain_inst = nc.sync.drain()
    wait_clock.add_sem_waits(
        drain_inst.ins, ScopedClock({None: tick_clock.global_clock})
    )
    si = drain_inst.ins.sync_info
    waits = list(si.on_wait) if si is not None else []
    if len(waits) > 1:
        drain_inst.ins.sync_info = mybir.SyncInfo(
            on_wait=waits[:1], on_update=list(si.on_update)
        )
        for w in waits[1:]:
            d2 = nc.sync.drain()
            d2.ins.sync_info = mybir.SyncInfo(on_wait=[w], on_update=[])
    nc.all_engine_barrier()
    popped = nc._tile_sem_poison_stack.pop()
    assert popped is self._sem_poison
    if not SKIP_CLEAR:
        # SKIP_CLEAR leaves the tile clock sems dirty at kernel end; the
        # ~6us of per-range dma_reset/sem_clear rounds only matter for
        # re-executing the same loaded NEFF (verified by the warm-run
        # output check in test.py).
        nc.clear_and_free_semaphores(list(self.sems.allocated().values()))
    nc.all_engine_barrier()


tile.TileContext._drain_and_barrier = _split_drain_and_barrier


def _split_multi_waits(nc):
    """Hoist extra sync waits onto same-engine NOPs (walrus: 1 wait/inst)."""
    for fn in nc.m.functions:
        for bb in fn.blocks:
            out = []
            changed = False
            for inst in bb.instructions:
                si = inst.sync_info
                if si is not None and len(si.on_wait) > 1:
                    waits = list(si.on_wait)
                    for w in waits[:-1]:
                        out.append(
                            mybir.InstNoOp(
                                name=nc.get_next_instruction_name(),
                                engine=inst.engine,
                                sync_info=mybir.SyncInfo(on_wait=[w], on_update=[]),
                                bass_nofuse=True,
                            )
                        )
                    inst.sync_info = mybir.SyncInfo(
                        on_wait=[waits[-1]], on_update=list(si.on_update)
                    )
                    changed = True
                out.append(inst)
            if changed:
                bb.instructions = out

# ---------------------------------------------------------------------------
# Problem constants (hardcoded per the task contract)
# ---------------------------------------------------------------------------
B, S, H, D = 2, 2048, 16, 64
NB = 32  # number of 64-wide blocks along S
N_CORES = 8
HPC = 4  # heads (flat b*H+h) per core
CHUNK = 16  # score col-blocks per PSUM chunk (2 PSUM banks of 8)
VSCALE = 1.0 / 16.0  # pre-scale on V and the ones column (fp16 range safety)
F16 = mybir.dt.float16
F32 = mybir.dt.float32
OT_DT = F16 if OUT_F16 else F32

# correction cases: per half in {0: live, 1: dead, 2: diagonal-triangle}
# case id for (top, bot); (0, 0) means no correction.
CASE_IDS = {}
for _tc in range(3):
    for _bc in range(3):
        if (_tc, _bc) != (0, 0):
            CASE_IDS[(_tc, _bc)] = len(CASE_IDS)
N_CASES = len(CASE_IDS)  # 8


def _match_pairs(mask):
    """Pair up the 32 k-blocks to maximize overlap of their active-q sets
    (greedy max-weight matching). Overlapping pairs make dense (dual) score
    columns, shrinking the union column count that drives QK/exp/PV work."""
    act = {
        kb: frozenset(qb for qb in range(kb, NB) if mask[qb, kb]) for kb in range(NB)
    }
    left = set(range(NB))
    pairs = []
    while left:
        best = None
        for i in left:
            for j in left:
                if j <= i:
                    continue
                sc = len(act[i] & act[j])
                if best is None or sc > best[0] or (sc == best[0] and (i, j) < best[1:]):
                    best = (sc, i, j)
        _, i, j = best
        pairs.append((i, j))
        left -= {i, j}
    pairs.sort()
    return pairs


def _head_schedule(mask, pairs, gap=GAP):
    """Columns of the S^T score layout for one head, PAIR-major.

    mask: [32, 32] bool. Active block (qb, kb) requires qb >= kb (block-level
    causal) and mask[qb, kb]. pairs: 16 (kb1, kb2) k-block pairs; pair t forms
    the 128-partition tile [K[kb1]; K[kb2]].

    Each column carries its correction case (dead halves / diagonal
    triangles); interior qb-gaps <= `gap` are bridged with fake fully-dead
    columns so QK/PV runs merge (fake columns are zeroed by their correction).
    """
    cols = []
    for t, (kb1, kb2) in enumerate(pairs):
        seq = []
        for qb in range(NB):
            top = qb >= kb1 and bool(mask[qb, kb1])
            bot = qb >= kb2 and bool(mask[qb, kb2])
            if top or bot:
                seq.append((qb, top, bot))
        ext = []
        for qb, top, bot in seq:
            if ext:
                prev_qb = ext[-1][0]
                if 1 < qb - prev_qb <= gap + 1:
                    for fqb in range(prev_qb + 1, qb):
                        ext.append((fqb, False, False))
            ext.append((qb, top, bot))
        for qb, top, bot in ext:
            tc = (2 if qb == kb1 else 0) if top else 1
            bc = (2 if qb == kb2 else 0) if bot else 1
            cols.append({"t": t, "qb": qb, "case": (tc, bc)})
    return cols


def _runs(chunk, key_consecutive, flags=None):
    """Split a chunk (list of (idx, col)) into affine matmul runs."""
    runs = []
    cur = []
    for item in chunk:
        if cur:
            _, pc = cur[-1]
            _, cc = item
            ok = key_consecutive(pc, cc) and (
                flags is None or flags(cc) == flags(pc)
            )
            if ok:
                cur.append(item)
                continue
            runs.append(cur)
        cur = [item]
    if cur:
        runs.append(cur)
    return runs


def _chunks_of(cols):
    """Cut cols into chunks of <= CHUNK (one PSUM bank each)."""
    return [cols[i : i + CHUNK] for i in range(0, len(cols), CHUNK)]


def build_program(schedules):
    """Build the Bass program for one core.

    schedules: list of HPC dicts {"pairs": [(kb1, kb2)]*16, "cols": [...]}.
    """
    nc = bass.Bass()
    # qt/kt/corr/irep carry the SAME data duplicated on partitions 0:64 and
    # 64:128 so QK/correction matmuls (contraction 64) can run on either
    # PE array row-group: two 8-col banks of a chunk execute CONCURRENTLY.
    # qt/kt are duplicated ON-CHIP (SBUF->SBUF DMA) to halve HBM traffic.
    qt = nc.declare_dram_parameter("qt", [HPC, 64, S], F16, isOutput=False)
    kt = nc.declare_dram_parameter("kt", [HPC, 64, S], F16, isOutput=False)
    va = nc.declare_dram_parameter("va", [HPC, 128, 16 * 65], F16, isOutput=False)
    corr = nc.declare_dram_parameter("corr", [128, N_CASES * 128], F16, isOutput=False)
    irep = nc.declare_dram_parameter("irep", [128, 8 * 64], F16, isOutput=False)
    ot = nc.declare_dram_parameter("ot", [HPC, 65, S], OT_DT, isOutput=True)

    with tile.TileContext(nc) as tc, ExitStack() as ctx:
        const = ctx.enter_context(tc.tile_pool(name="const", bufs=1))
        # per-head input pools, double-buffered: head s+2's DMA waits until
        # head s's tiles are fully consumed, so the DMA fabric prioritizes
        # the first heads instead of spreading bandwidth over all four.
        qtp = ctx.enter_context(tc.tile_pool(name="qtp", bufs=2))
        ktp = ctx.enter_context(tc.tile_pool(name="ktp", bufs=4))  # 2 heads x (ka, kb)
        vap = ctx.enter_context(tc.tile_pool(name="vap", bufs=2))
        pts = ctx.enter_context(tc.tile_pool(name="pts", bufs=DEPTH + 2))
        outp = ctx.enter_context(tc.tile_pool(name="outp", bufs=2))
        psS = ctx.enter_context(tc.tile_pool(name="psS", bufs=2, space="PSUM"))
        psO = ctx.enter_context(tc.tile_pool(name="psO", bufs=1, space="PSUM"))

        corr_t = const.tile([128, N_CASES * 128], F16, tag="corr")
        nc.sync.dma_start(out=corr_t[:], in_=corr[:])
        irep_t = const.tile([128, 8 * 64], F16, tag="irep")
        nc.sync.dma_start(out=irep_t[:], in_=irep[:])
        zeros = const.tile([128, 128], F16, tag="zeros")
        nc.vector.memset(zeros[:], 0.0)
        # explicit zero bias for exp: a float bias would register a const-AP,
        # whose database load costs a TENSOR_LOAD barrier round in the preamble
        bias0 = const.tile([128, 1], F32, tag="bias0")
        nc.vector.memset(bias0[:], 0.0)

        if WARMUP:
            wps = psS.tile([128, 64 * CHUNK], F32, tag="ps")
            for _ in range(WARMUP):
                nc.tensor.matmul(
                    wps[:, 0:128],
                    lhsT=zeros[:, 0:128],
                    rhs=zeros[:, 0:128],
                    start=True,
                    stop=True,
                )

        qts, kts, vas = {}, {}, {}

        def load_head(s):
            # Duplicate Q^T/K^T onto partitions 64:128 (for row-group-1
            # matmuls) by reading HBM twice CONCURRENTLY: a chained
            # SBUF->SBUF dup costs two serialized DMA-completion sem hops
            # (~0.9us each) on the head-0 critical path. K^T is split into
            # two half-tiles (pairs 0-7 / 8-15): the t-major schedule only
            # touches the second half mid-head, so the head's first chunks
            # wait on ~40% fewer critical bytes.
            qs = qtp.tile([128, S], F16, tag="qt")
            ka = ktp.tile([128, S // 2], F16, tag="ka")
            kb = ktp.tile([128, S // 2], F16, tag="kb")
            vs = vap.tile([128, 16 * 65], F16, tag="va")
            hs2 = S // 2
            nc.sync.dma_start(out=qs[0:64, :], in_=qt[s])
            nc.sync.dma_start(out=qs[64:128, :], in_=qt[s])
            nc.sync.dma_start(out=ka[0:64, :], in_=kt[s, :, 0:hs2])
            nc.sync.dma_start(out=ka[64:128, :], in_=kt[s, :, 0:hs2])
            nc.sync.dma_start(out=vs[:], in_=va[s])
            nc.sync.dma_start(out=kb[0:64, :], in_=kt[s, :, hs2:S])
            nc.sync.dma_start(out=kb[64:128, :], in_=kt[s, :, hs2:S])
            qts[s], kts[s], vas[s] = qs, (ka, kb), vs

        # head 0 fully first: its compute start gates everything, and the
        # in-order DMA queues would otherwise split bandwidth with head 1
        load_head(0)
        load_head(1)

        # Global software pipeline over all (head, chunk) pairs: PV emission
        # runs DEPTH chunks behind QK/exp, crossing head boundaries so
        # neither the PE nor the ACT engine drains between heads.
        heads = []
        for s in range(HPC):
            oT = psO.tile([128, S], F32, tag="psO")
            heads.append(
                {
                    "s": s,
                    "oT": oT,
                    "opened": False,
                    "left": len(_chunks_of(schedules[s]["cols"])),
                }
            )

        def open_oT(hd):
            # Zero-open each O^T bank with the bank's only start=True matmul:
            # start=True marks the whole 2KB bank pending-zero, so PV's
            # start=False writes zero-fill on first touch and accumulates
            # after. Deferred to the head's first PV so the PE front-runs QK
            # while the previous head's output copy still holds the banks.
            for g in range(NB // 8):
                nc.tensor.matmul(
                    hd["oT"][0:65, 512 * g : 512 * g + 1],
                    lhsT=zeros[:, 0:65],
                    rhs=zeros[:, 0:1],
                    start=True,
                    stop=False,
                    skip_group_check=True,
                )
            hd["opened"] = True

        def emit_copy(hd):
            # Evacuate O^T on DVE + GPSIMD in parallel (both idle engines);
            # the ACT engine stays dedicated to exp, its critical path.
            s = hd["s"]
            o_sb = outp.tile([65, S], OT_DT, tag="o")
            hs2 = S // 2
            nc.vector.tensor_copy(out=o_sb[:, 0:hs2], in_=hd["oT"][0:65, 0:hs2])
            nc.sync.dma_start(out=ot[s, :, 0:hs2], in_=o_sb[:, 0:hs2])
            nc.vector.tensor_copy(out=o_sb[:, hs2:S], in_=hd["oT"][0:65, hs2:S])
            nc.sync.dma_start(out=ot[s, :, hs2:S], in_=o_sb[:, hs2:S])

        pend = []  # (pt_tile, chunk, head-record)

        def emit_pv():
            pt, chunk, hd = pend.pop(0)
            if not hd["opened"]:
                open_oT(hd)
            s = hd["s"]
            pv = _runs(
                chunk,
                key_consecutive=lambda p, c: p["t"] == c["t"]
                and c["qb"] == p["qb"] + 1
                and c["qb"] % 8 != 0,  # O^T bank boundary
            )
            for run in pv:
                i0, rc = run[0]
                n = len(run)
                nc.tensor.matmul(
                    hd["oT"][0:65, 64 * rc["qb"] : 64 * (rc["qb"] + n)],
                    lhsT=vas[s][:, 65 * rc["t"] : 65 * (rc["t"] + 1)],
                    rhs=pt[:, 64 * i0 : 64 * (i0 + n)],
                    start=False,
                    stop=True,
                    skip_group_check=True,
                )
            hd["left"] -= 1
            if hd["left"] == 0:
                emit_copy(hd)

        work = []
        for s in range(HPC):
            for chunk_cols in _chunks_of(schedules[s]["cols"]):
                work.append((s, chunk_cols))

        loaded = 2
        for s, chunk_cols in work:
            chunk = list(enumerate(chunk_cols))
            L = len(chunk)
            ps = psS.tile([128, 64 * CHUNK], F32, tag="ps")

            # Per 8-col PSUM bank: zero-open (start=True, 1 col), QK runs
            # and correction runs with start=False (first touch
            # write-fills). Bank b's matmuls contract on array row-group
            # 64*b using the duplicated operand partitions; the two banks'
            # emissions are INTERLEAVED so consecutive PE instructions hit
            # disjoint row-groups and execute CONCURRENTLY on the array.
            lanes = [[], []]
            for b in range(2):
                sub = [it for it in chunk if it[0] // 8 == b]
                if not sub:
                    continue
                rg = 64 * b

                def _open(b=b, rg=rg, ps=ps):
                    nc.tensor.matmul(
                        ps[:, 512 * b : 512 * b + 1],
                        lhsT=zeros[rg : rg + 64, 0:128],
                        rhs=zeros[rg : rg + 64, 0:1],
                        start=True,
                        stop=False,
                        skip_group_check=True,
                    )

                lanes[b].append(_open)
                qk = _runs(
                    sub,
                    key_consecutive=lambda p, c: p["t"] == c["t"]
                    and c["qb"] == p["qb"] + 1,
                )
                for run in qk:
                    i0, rc = run[0]
                    n = len(run)

                    def _qk(i0=i0, rc=rc, n=n, rg=rg, ps=ps, s=s):
                        kh = kts[s][rc["t"] // 8]
                        t2 = rc["t"] % 8
                        nc.tensor.matmul(
                            ps[:, 64 * i0 : 64 * (i0 + n)],
                            lhsT=kh[rg : rg + 64, 128 * t2 : 128 * (t2 + 1)],
                            rhs=qts[s][
                                rg : rg + 64, 64 * rc["qb"] : 64 * (rc["qb"] + n)
                            ],
                            start=False,
                            stop=True,
                            skip_group_check=True,
                        )

                    lanes[b].append(_qk)
                # Corrections: one matmul per equal-case run adds the
                # -2000 mask (stationary = case pattern, moving = I64s).
                cr = _runs(
                    sub,
                    key_consecutive=lambda p, c: True,
                    flags=lambda c: c["case"],
                )
                for run in cr:
                    i0, rc = run[0]
                    if rc["case"] == (0, 0):
                        continue
                    n = len(run)
                    cid = CASE_IDS[rc["case"]]

                    def _cr(i0=i0, n=n, cid=cid, rg=rg, ps=ps):
                        nc.tensor.matmul(
                            ps[:, 64 * i0 : 64 * (i0 + n)],
                            lhsT=corr_t[rg : rg + 64, 128 * cid : 128 * (cid + 1)],
                            rhs=irep_t[rg : rg + 64, 0 : 64 * n],
                            start=False,
                            stop=True,
                            skip_group_check=True,
                        )

                    lanes[b].append(_cr)
            for i2 in range(max(len(lanes[0]), len(lanes[1]))):
                for b in range(2):
                    if i2 < len(lanes[b]):
                        lanes[b][i2]()

            pt = pts.tile([128, 64 * CHUNK], F16, tag="pt")
            nc.scalar.activation(
                out=pt[:, : 64 * L],
                in_=ps[:, : 64 * L],
                func=mybir.ActivationFunctionType.Exp,
                bias=bias0[:],
                scale=0.125,
            )
            pend.append((pt, chunk, heads[s]))
            if len(pend) > DEPTH:
                emit_pv()
            # prefetch: issue head s+2's HBM loads after head s's first chunk
            if s + 2 == loaded + 0 and loaded < HPC:
                load_head(loaded)
                loaded += 1
        while pend:
            emit_pv()

    _split_multi_waits(nc)
    return nc


def _assignment(schedules):
    """Balanced head->core assignment: greedy longest-first onto the least
    loaded core with capacity HPC. Cost = column count (drives every stage).
    Returns list of N_CORES lists of flat head ids."""
    costs = [(len(schedules[g]["cols"]), g) for g in range(B * H)]
    costs.sort(reverse=True)
    loads = [0.0] * N_CORES
    slots = [[] for _ in range(N_CORES)]
    for cost, g in costs:
        c = min(
            (c for c in range(N_CORES) if len(slots[c]) < HPC),
            key=lambda c: (loads[c], c),
        )
        slots[c].append(g)
        loads[c] += cost
    return [sorted(sl) for sl in slots]


def _corr_patterns():
    """[64, N_CASES*128] fp16: stationary A with A.T @ I64 = U_case, where
    U_case [128, 64] holds the -2000 mask for (top, bot) case halves."""
    strict = np.tril(np.full((64, 64), NEG, np.float32), k=-1)  # kl > ql
    dead = np.full((64, 64), NEG, np.float32)
    live = np.zeros((64, 64), np.float32)
    half = {0: live, 1: dead, 2: strict}
    out = np.zeros((64, N_CASES * 128), np.float16)
    for (tc, bc), cid in CASE_IDS.items():
        U = np.concatenate([half[tc], half[bc]], axis=0)  # [128, 64]
        out[:, 128 * cid : 128 * (cid + 1)] = U.T.astype(np.float16)
    return np.ascontiguousarray(out)


def _prep_inputs(q, k, v, schedules):
    """Per-core input arrays keyed as the programs expect."""
    assign = _assignment(schedules)
    # flat head g = b*H + h
    qt_all = np.ascontiguousarray(
        q.transpose(0, 2, 3, 1).reshape(B * H, D, S).astype(np.float16)
    )
    kt_nat = k.transpose(0, 2, 3, 1).reshape(B * H, D, S).astype(np.float16)
    kt_nat = kt_nat.reshape(B * H, D, NB, 64)
    kt_all = np.empty_like(kt_nat)
    for g in range(B * H):
        order = [kb for p in schedules[g]["pairs"] for kb in p]
        kt_all[g] = kt_nat[g][:, order, :]
    kt_all = np.ascontiguousarray(kt_all.reshape(B * H, D, S))
    v_aug = np.concatenate([v, np.ones((B, S, H, 1), v.dtype)], axis=3)  # [B,S,H,65]
    v_aug = v_aug * VSCALE
    vb_all = v_aug.transpose(0, 2, 1, 3).reshape(B * H, NB, 64, 65)  # [g, kb, 64, 65]
    # va[g]: per pair t, rows 0:64 = V[kb1] block, rows 64:128 = V[kb2]
    va_all = np.zeros((B * H, 128, 16 * 65), np.float16)
    for g in range(B * H):
        for t, (kb1, kb2) in enumerate(schedules[g]["pairs"]):
            va_all[g, 0:64, 65 * t : 65 * (t + 1)] = vb_all[g, kb1]
            va_all[g, 64:128, 65 * t : 65 * (t + 1)] = vb_all[g, kb2]
    corr_half = _corr_patterns()
    corr_full = np.ascontiguousarray(np.concatenate([corr_half, corr_half], axis=0))
    irep_half = np.tile(np.eye(64, dtype=np.float16), (1, 8))
    irep_full = np.ascontiguousarray(np.concatenate([irep_half, irep_half], axis=0))
    in_maps = []
    for c in range(N_CORES):
        gs = assign[c]
        in_maps.append(
            {
                "qt": np.ascontiguousarray(qt_all[gs]),
                "kt": np.ascontiguousarray(kt_all[gs]),
                "va": np.ascontiguousarray(va_all[gs]),
                "corr": corr_full,
                "irep": irep_full,
            }
        )
    return in_maps


def _schedules(block_mask):
    """Per flat head: k-block pairing + column schedule."""
    masks_all = np.asarray(block_mask).reshape(B * H, NB, NB)
    scheds = []
    for g in range(B * H):
        if PAIRING == "match":
            pairs = _match_pairs(masks_all[g])
        else:
            pairs = [(2 * t, 2 * t + 1) for t in range(NB // 2)]
        scheds.append({"pairs": pairs, "cols": _head_schedule(masks_all[g], pairs)})
    return scheds


_PROG_CACHE = {}


def _get_programs(block_mask, schedules):
    key = np.asarray(block_mask).tobytes()
    if key not in _PROG_CACHE:
        assign = _assignment(schedules)
        _PROG_CACHE[key] = [
            build_program([schedules[g] for g in assign[c]]) for c in range(N_CORES)
        ]
    return _PROG_CACHE[key]


def run_cores(ncs, in_maps, trace=False):
    """Run the 8 per-core programs concurrently on the 8 devices."""
    import jax

    devs = jax.devices()
    results = [None] * N_CORES
    errs = [None] * N_CORES

    def _run(c):
        try:
            with jax.default_device(devs[c]):
                r = run_bass_kernel_spmd(
                    ncs[c], [in_maps[c]], core_ids=[0], trace=trace and c == 0
                )
                results[c] = r
        except Exception as e:  # noqa: BLE001
            errs[c] = e

    threads = [threading.Thread(target=_run, args=(c,)) for c in range(N_CORES)]
    for t in threads:
        t.start()
    for t in threads:
        t.join()
    for c, e in enumerate(errs):
        if e is not None:
            raise RuntimeError(f"core {c} failed") from e
    return results


def kernel(q, k, v, block_mask):
    q = np.asarray(q, dtype=np.float32)
    k = np.asarray(k, dtype=np.float32)
    v = np.asarray(v, dtype=np.float32)
    block_mask = np.asarray(block_mask).astype(bool)

    schedules = _schedules(block_mask)
    assign = _assignment(schedules)
    in_maps = _prep_inputs(q, k, v, schedules)
    ncs = _get_programs(block_mask, schedules)
    results = run_cores(ncs, in_maps)

    out = np.empty((B, S, H, D), np.float32)
    for c in range(N_CORES):
        ot = results[c].results[0]["ot"]  # [HPC, 65, S]
        for s in range(HPC):
            g = assign[c][s]
            b, h = divmod(g, H)
            o_un = ot[s, :D, :].astype(np.float32)  # [D, S] unnormalized (x1/16)
            l = ot[s, D, :].astype(np.float32)  # [S] denominator (x1/16)
            out[b, :, h, :] = (o_un / l[None, :]).T
    return out


# revision 48
# speedup vs baseline: 1.0377x; 1.0377x over previous
"""Block-sparse attention kernel for Trainium2 (8 NeuronCores).

Problem: B=2, S=2048, H=16, Dqk=Dv=64, 64x64 block mask (30% + forced diag),
AND causal. out = softmax(mask(QK^T/8)) @ V.

Strategy
--------
- Shard the 32 (batch, head) pairs across 8 cores, 4 heads per core,
  balanced by per-head schedule cost.
- Each core gets its OWN Bass program with the sparse block schedule baked in
  from its heads' block masks (compiled at call time, run concurrently on the
  8 axon devices).
- Per head, scores are computed TRANSPOSED (S^T[k, q]) so that P^T = exp(S^T)
  lands in SBUF in exactly the layout PV needs (k on partitions) — no on-chip
  transposes anywhere:
    * host supplies Q^T and K^T as [64(d), 2048(s)] fp16, V as [128, 16*65]
      fp16 "v-pair" tiles [V[kb1]; V[kb2]] with a ones column (col 64); V and
      the ones column are pre-scaled by 1/16 so the fp16 outputs can't
      overflow (the host ratio o_un/l is scale-invariant).
    * k-blocks are paired by greedy max-overlap matching of their active-q
      sets, so paired columns are dense (fewer dead halves).
    * per score chunk (8 columns = 1 PSUM bank): one 1-col start=True matmul
      zero-opens the bank, QK runs (lhsT=K^T pair [64,128], rhs=Q^T qb-run)
      accumulate, then CORRECTION matmuls add -2000 onto masked regions
      (dead 64x64 halves and strict-upper triangles of diagonal blocks) so
      exp maps them to exact zeros. All masking lives on the PE — the
      DVE/GPSIMD never touch the score path.
    * exp: one ACT op per chunk (scale=1/8 fused), fp16 out.
    * PV: matmul(lhsT=[V|1/16] pair [128,65], rhs=P^T run) accumulating
      O^T[65, 2048] in PSUM across k-pairs (banks zero-opened up front).
    * O^T (unnormalized, with row 64 = softmax denominator l/16) is copied to
      SBUF fp16 in two pipelined pieces on the DVE (the ACT engine stays
      dedicated to exp) and DMA'd out; the host divides and transposes back.
- Corrections use constant stationaries: for each (top, bottom) mask case a
  [64,128] pattern P_case with P_case.T @ I64 = the -2000 mask; the moving
  operand is a repeated-identity tile, so one matmul corrects a whole run of
  equal-case columns.
- The chunk chain is PE(open+QK+corr) -> ACT(exp) -> PE(PV): PV emission is
  software-pipelined DEPTH chunks behind QK — across head boundaries — so
  the in-order PE queue never stalls on exp. A zeros warm-up stream during
  the input-DMA wait promotes the HAM clock gate to 8/8 (2.4 GHz) before
  compute starts; micro-idle-heavy schedules sit at 1.2 GHz.
- A chunk is 16 columns = 2 PSUM banks; bank A's QK/corr matmuls contract on
  PE array rows 0:63 and bank B's on rows 64:127 (tile row-groups, operands
  duplicated on-chip onto partitions 64:128), emissions interleaved so the
  two banks execute CONCURRENTLY on the half-used (K=64) array.
- Schedule is pair-major (t-major): all columns of one k-pair are emitted
  together so consecutive matmuls share stationary weights.
- Inputs stream per head through double-buffered pools: head s+2's DMA can
  only start once head s is consumed, so the fabric gives early heads full
  bandwidth and compute starts ~2 heads of DMA earlier.
- Softmax uses no running max: inputs are N(0,1) so scores/8 stay in a range
  where exp() is safely finite (exp(~7) ~ 1e3).
"""

import os
import threading
from contextlib import ExitStack

import numpy as np

import concourse.bass as bass
import concourse.tile as tile
from concourse import mybir
from concourse.bass_utils import run_bass_kernel_spmd
from concourse.vector_clock import ScopedClock

# ---------------------------------------------------------------------------
# Tunables (env-overridable for A/B testing)
# ---------------------------------------------------------------------------
PAIRING = os.environ.get("BSA_PAIRING", "match")  # match | adjacent
GAP = int(os.environ.get("BSA_GAP", "0"))
OUT_F16 = os.environ.get("BSA_OUT_F16", "1") == "1"
SPLIT_COPY = os.environ.get("BSA_SPLIT_COPY", "1") == "1"
WARMUP = int(os.environ.get("BSA_WARMUP", "48"))  # 128-row dummy matmuls: ends
# right as head 0's inputs land, having held the PE busy >3.4us to promote
# the HAM clock gate before real compute starts
DEPTH = int(os.environ.get("BSA_DEPTH", "2"))  # PV pipeline depth (chunks)
SKIP_CLEAR = os.environ.get("BSA_SKIP_CLEAR", "0") == "1"
NEG = -2000.0  # score offset for masked regions: exp((s-2000)/8) == 0

# ----------------------------------------------------------------------------
# Workaround: the installed walrus rejects instructions with more than one
# sync wait. Tile's kernel-tail drain attaches every outstanding clock sem to
# one Drain instruction; split them one wait per Drain.
# ----------------------------------------------------------------------------


def _split_drain_and_barrier(self, tick_clock, wait_clock):
    nc = self.nc
    drain_inst = nc.sync.drain()
    wait_clock.add_sem_waits(
        drain_inst.ins, ScopedClock({None: tick_clock.global_clock})
    )
    si = drain_inst.ins.sync_info
    waits = list(si.on_wait) if si is not None else []
    if len(waits) > 1:
        drain_inst.ins.sync_info = mybir.SyncInfo(
            on_wait=waits[:1], on_update=list(si.on_update)
        )
        for w in waits[1:]:
            d2 = nc.sync.drain()
            d2.ins.sync_info = mybir.SyncInfo(on_wait=[w], on_update=[])
    nc.all_engine_barrier()
    popped = nc._tile_sem_poison_stack.pop()
    assert popped is self._sem_poison
    if not SKIP_CLEAR:
        # SKIP_CLEAR leaves the tile clock sems dirty at kernel end; the
        # ~6us of per-range dma_reset/sem_clear rounds only matter for
        # re-executing the same loaded NEFF (verified by the warm-run
        # output check in test.py).
        nc.clear_and_free_semaphores(list(self.sems.allocated().values()))
    nc.all_engine_barrier()


tile.TileContext._drain_and_barrier = _split_drain_and_barrier


def _split_multi_waits(nc):
    """Hoist extra sync waits onto same-engine NOPs (walrus: 1 wait/inst)."""
    for fn in nc.m.functions:
        for bb in fn.blocks:
            out = []
            changed = False
            for inst in bb.instructions:
                si = inst.sync_info
                if si is not None and len(si.on_wait) > 1:
                    waits = list(si.on_wait)
                    for w in waits[:-1]:
                        out.append(
                            mybir.InstNoOp(
                                name=nc.get_next_instruction_name(),
                                engine=inst.engine,
                                sync_info=mybir.SyncInfo(on_wait=[w], on_update=[]),
                                bass_nofuse=True,
                            )
                        )
                    inst.sync_info = mybir.SyncInfo(
                        on_wait=[waits[-1]], on_update=list(si.on_update)
                    )
                    changed = True
                out.append(inst)
            if changed:
                bb.instructions = out

# ---------------------------------------------------------------------------
# Problem constants (hardcoded per the task contract)
# ---------------------------------------------------------------------------
B, S, H, D = 2, 2048, 16, 64
NB = 32  # number of 64-wide blocks along S
N_CORES = 8
HPC = 4  # heads (flat b*H+h) per core
CHUNK = 16  # score col-blocks per PSUM chunk (2 PSUM banks of 8)
VSCALE = 1.0 / 16.0  # pre-scale on V and the ones column (fp16 range safety)
F16 = mybir.dt.float16
F32 = mybir.dt.float32
OT_DT = F16 if OUT_F16 else F32

# correction cases: per half in {0: live, 1: dead, 2: diagonal-triangle}
# case id for (top, bot); (0, 0) means no correction.
CASE_IDS = {}
for _tc in range(3):
    for _bc in range(3):
        if (_tc, _bc) != (0, 0):
            CASE_IDS[(_tc, _bc)] = len(CASE_IDS)
N_CASES = len(CASE_IDS)  # 8


def _match_pairs(mask):
    """Pair up the 32 k-blocks to maximize overlap of their active-q sets
    (greedy max-weight matching). Overlapping pairs make dense (dual) score
    columns, shrinking the union column count that drives QK/exp/PV work."""
    act = {
        kb: frozenset(qb for qb in range(kb, NB) if mask[qb, kb]) for kb in range(NB)
    }
    left = set(range(NB))
    pairs = []
    while left:
        best = None
        for i in left:
            for j in left:
                if j <= i:
                    continue
                sc = len(act[i] & act[j])
                if best is None or sc > best[0] or (sc == best[0] and (i, j) < best[1:]):
                    best = (sc, i, j)
        _, i, j = best
        pairs.append((i, j))
        left -= {i, j}
    pairs.sort()
    return pairs


def _head_schedule(mask, pairs, gap=GAP):
    """Columns of the S^T score layout for one head, PAIR-major.

    mask: [32, 32] bool. Active block (qb, kb) requires qb >= kb (block-level
    causal) and mask[qb, kb]. pairs: 16 (kb1, kb2) k-block pairs; pair t forms
    the 128-partition tile [K[kb1]; K[kb2]].

    Each column carries its correction case (dead halves / diagonal
    triangles); interior qb-gaps <= `gap` are bridged with fake fully-dead
    columns so QK/PV runs merge (fake columns are zeroed by their correction).
    """
    cols = []
    for t, (kb1, kb2) in enumerate(pairs):
        seq = []
        for qb in range(NB):
            top = qb >= kb1 and bool(mask[qb, kb1])
            bot = qb >= kb2 and bool(mask[qb, kb2])
            if top or bot:
                seq.append((qb, top, bot))
        ext = []
        for qb, top, bot in seq:
            if ext:
                prev_qb = ext[-1][0]
                if 1 < qb - prev_qb <= gap + 1:
                    for fqb in range(prev_qb + 1, qb):
                        ext.append((fqb, False, False))
            ext.append((qb, top, bot))
        for qb, top, bot in ext:
            tc = (2 if qb == kb1 else 0) if top else 1
            bc = (2 if qb == kb2 else 0) if bot else 1
            cols.append({"t": t, "qb": qb, "case": (tc, bc)})
    return cols


def _runs(chunk, key_consecutive, flags=None):
    """Split a chunk (list of (idx, col)) into affine matmul runs."""
    runs = []
    cur = []
    for item in chunk:
        if cur:
            _, pc = cur[-1]
            _, cc = item
            ok = key_consecutive(pc, cc) and (
                flags is None or flags(cc) == flags(pc)
            )
            if ok:
                cur.append(item)
                continue
            runs.append(cur)
        cur = [item]
    if cur:
        runs.append(cur)
    return runs


def _chunks_of(cols):
    """Cut cols into chunks of <= CHUNK (one PSUM bank each)."""
    return [cols[i : i + CHUNK] for i in range(0, len(cols), CHUNK)]


def build_program(schedules):
    """Build the Bass program for one core.

    schedules: list of HPC dicts {"pairs": [(kb1, kb2)]*16, "cols": [...]}.
    """
    nc = bass.Bass()
    # qt/kt/corr/irep carry the SAME data duplicated on partitions 0:64 and
    # 64:128 so QK/correction matmuls (contraction 64) can run on either
    # PE array row-group: two 8-col banks of a chunk execute CONCURRENTLY.
    # qt/kt are duplicated ON-CHIP (SBUF->SBUF DMA) to halve HBM traffic.
    qt = nc.declare_dram_parameter("qt", [HPC, 64, S], F16, isOutput=False)
    kt = nc.declare_dram_parameter("kt", [HPC, 64, S], F16, isOutput=False)
    va = nc.declare_dram_parameter("va", [HPC, 128, 16 * 65], F16, isOutput=False)
    corr = nc.declare_dram_parameter("corr", [128, N_CASES * 128], F16, isOutput=False)
    irep = nc.declare_dram_parameter("irep", [128, 8 * 64], F16, isOutput=False)
    ot = nc.declare_dram_parameter("ot", [HPC, 65, S], OT_DT, isOutput=True)

    with tile.TileContext(nc) as tc, ExitStack() as ctx:
        const = ctx.enter_context(tc.tile_pool(name="const", bufs=1))
        # per-head input pools, double-buffered: head s+2's DMA waits until
        # head s's tiles are fully consumed, so the DMA fabric prioritizes
        # the first heads instead of spreading bandwidth over all four.
        qtp = ctx.enter_context(tc.tile_pool(name="qtp", bufs=2))
        ktp = ctx.enter_context(tc.tile_pool(name="ktp", bufs=4))  # 2 heads x (ka, kb)
        vap = ctx.enter_context(tc.tile_pool(name="vap", bufs=2))
        pts = ctx.enter_context(tc.tile_pool(name="pts", bufs=DEPTH + 2))
        outp = ctx.enter_context(tc.tile_pool(name="outp", bufs=2))
        psS = ctx.enter_context(tc.tile_pool(name="psS", bufs=2, space="PSUM"))
        psO = ctx.enter_context(tc.tile_pool(name="psO", bufs=1, space="PSUM"))

        corr_t = const.tile([128, N_CASES * 128], F16, tag="corr")
        nc.sync.dma_start(out=corr_t[:], in_=corr[:])
        irep_t = const.tile([128, 8 * 64], F16, tag="irep")
        nc.sync.dma_start(out=irep_t[:], in_=irep[:])
        zeros = const.tile([128, 128], F16, tag="zeros")
        nc.vector.memset(zeros[:], 0.0)
        # explicit zero bias for exp: a float bias would register a const-AP,
        # whose database load costs a TENSOR_LOAD barrier round in the preamble
        bias0 = const.tile([128, 1], F32, tag="bias0")
        nc.vector.memset(bias0[:], 0.0)

        if WARMUP:
            wps = psS.tile([128, 64 * CHUNK], F32, tag="ps")
            for _ in range(WARMUP):
                nc.tensor.matmul(
                    wps[:, 0:128],
                    lhsT=zeros[:, 0:128],
                    rhs=zeros[:, 0:128],
                    start=True,
                    stop=True,
                )

        qts, kts, vas = {}, {}, {}

        def load_head(s):
            # Duplicate Q^T/K^T onto partitions 64:128 (for row-group-1
            # matmuls) by reading HBM twice CONCURRENTLY: a chained
            # SBUF->SBUF dup costs two serialized DMA-completion sem hops
            # (~0.9us each) on the head-0 critical path. K^T is split into
            # two half-tiles (pairs 0-7 / 8-15): the t-major schedule only
            # touches the second half mid-head, so the head's first chunks
            # wait on ~40% fewer critical bytes.
            qs = qtp.tile([128, S], F16, tag="qt")
            ka = ktp.tile([128, S // 2], F16, tag="ka")
            kb = ktp.tile([128, S // 2], F16, tag="kb")
            vs = vap.tile([128, 16 * 65], F16, tag="va")
            hs2 = S // 2
            nc.sync.dma_start(out=qs[0:64, :], in_=qt[s])
            nc.sync.dma_start(out=qs[64:128, :], in_=qt[s])
            nc.sync.dma_start(out=ka[0:64, :], in_=kt[s, :, 0:hs2])
            nc.sync.dma_start(out=ka[64:128, :], in_=kt[s, :, 0:hs2])
            nc.sync.dma_start(out=vs[:], in_=va[s])
            nc.sync.dma_start(out=kb[0:64, :], in_=kt[s, :, hs2:S])
            nc.sync.dma_start(out=kb[64:128, :], in_=kt[s, :, hs2:S])
            qts[s], kts[s], vas[s] = qs, (ka, kb), vs

        # head 0 fully first: its compute start gates everything, and the
        # in-order DMA queues would otherwise split bandwidth with head 1
        load_head(0)
        load_head(1)

        # Global software pipeline over all (head, chunk) pairs: PV emission
        # runs DEPTH chunks behind QK/exp, crossing head boundaries so
        # neither the PE nor the ACT engine drains between heads.
        heads = []
        for s in range(HPC):
            oT = psO.tile([128, S], F32, tag="psO")
            heads.append(
                {
                    "s": s,
                    "oT": oT,
                    "opened": False,
                    "left": len(_chunks_of(schedules[s]["cols"])),
                }
            )

        def open_oT(hd):
            # Zero-open each O^T bank with the bank's only start=True matmul:
            # start=True marks the whole 2KB bank pending-zero, so PV's
            # start=False writes zero-fill on first touch and accumulates
            # after. Deferred to the head's first PV so the PE front-runs QK
            # while the previous head's output copy still holds the banks.
            for g in range(NB // 8):
                nc.tensor.matmul(
                    hd["oT"][0:65, 512 * g : 512 * g + 1],
                    lhsT=zeros[:, 0:65],
                    rhs=zeros[:, 0:1],
                    start=True,
                    stop=False,
                    skip_group_check=True,
                )
            hd["opened"] = True

        def emit_copy(hd):
            # Evacuate O^T on DVE + GPSIMD in parallel (both idle engines);
            # the ACT engine stays dedicated to exp, its critical path.
            s = hd["s"]
            o_sb = outp.tile([65, S], OT_DT, tag="o")
            hs2 = S // 2
            nc.vector.tensor_copy(out=o_sb[:, 0:hs2], in_=hd["oT"][0:65, 0:hs2])
            nc.sync.dma_start(out=ot[s, :, 0:hs2], in_=o_sb[:, 0:hs2])
            nc.vector.tensor_copy(out=o_sb[:, hs2:S], in_=hd["oT"][0:65, hs2:S])
            nc.sync.dma_start(out=ot[s, :, hs2:S], in_=o_sb[:, hs2:S])

        pend = []  # (pt_tile, chunk, head-record)

        def emit_pv():
            pt, chunk, hd = pend.pop(0)
            if not hd["opened"]:
                open_oT(hd)
            s = hd["s"]
            pv = _runs(
                chunk,
                key_consecutive=lambda p, c: p["t"] == c["t"]
                and c["qb"] == p["qb"] + 1
                and c["qb"] % 8 != 0,  # O^T bank boundary
            )
            for run in pv:
                i0, rc = run[0]
                n = len(run)
                nc.tensor.matmul(
                    hd["oT"][0:65, 64 * rc["qb"] : 64 * (rc["qb"] + n)],
                    lhsT=vas[s][:, 65 * rc["t"] : 65 * (rc["t"] + 1)],
                    rhs=pt[:, 64 * i0 : 64 * (i0 + n)],
                    start=False,
                    stop=True,
                    skip_group_check=True,
                )
            hd["left"] -= 1
            if hd["left"] == 0:
                emit_copy(hd)

        work = []
        for s in range(HPC):
            for chunk_cols in _chunks_of(schedules[s]["cols"]):
                work.append((s, chunk_cols))

        loaded = 2
        for s, chunk_cols in work:
            chunk = list(enumerate(chunk_cols))
            L = len(chunk)
            ps = psS.tile([128, 64 * CHUNK], F32, tag="ps")

            # Per 8-col PSUM bank: zero-open (start=True, 1 col), QK runs
            # and correction runs with start=False (first touch
            # write-fills). Bank b's matmuls contract on array row-group
            # 64*b using the duplicated operand partitions; the two banks'
            # emissions are INTERLEAVED so consecutive PE instructions hit
            # disjoint row-groups and execute CONCURRENTLY on the array.
            lanes = [[], []]
            for b in range(2):
                sub = [it for it in chunk if it[0] // 8 == b]
                if not sub:
                    continue
                rg = 64 * b

                def _open(b=b, rg=rg, ps=ps):
                    nc.tensor.matmul(
                        ps[:, 512 * b : 512 * b + 1],
                        lhsT=zeros[rg : rg + 64, 0:128],
                        rhs=zeros[rg : rg + 64, 0:1],
                        start=True,
                        stop=False,
                        skip_group_check=True,
                    )

                lanes[b].append(_open)
                qk = _runs(
                    sub,
                    key_consecutive=lambda p, c: p["t"] == c["t"]
                    and c["qb"] == p["qb"] + 1,
                )
                for run in qk:
                    i0, rc = run[0]
                    n = len(run)

                    def _qk(i0=i0, rc=rc, n=n, rg=rg, ps=ps, s=s):
                        kh = kts[s][rc["t"] // 8]
                        t2 = rc["t"] % 8
                        nc.tensor.matmul(
                            ps[:, 64 * i0 : 64 * (i0 + n)],
                            lhsT=kh[rg : rg + 64, 128 * t2 : 128 * (t2 + 1)],
                            rhs=qts[s][
                                rg : rg + 64, 64 * rc["qb"] : 64 * (rc["qb"] + n)
                            ],
                            start=False,
                            stop=True,
                            skip_group_check=True,
                        )

                    lanes[b].append(_qk)
                # Corrections: one matmul per equal-case run adds the
                # -2000 mask (stationary = case pattern, moving = I64s).
                cr = _runs(
                    sub,
                    key_consecutive=lambda p, c: True,
                    flags=lambda c: c["case"],
                )
                for run in cr:
                    i0, rc = run[0]
                    if rc["case"] == (0, 0):
                        continue
                    n = len(run)
                    cid = CASE_IDS[rc["case"]]

                    def _cr(i0=i0, n=n, cid=cid, rg=rg, ps=ps):
                        nc.tensor.matmul(
                            ps[:, 64 * i0 : 64 * (i0 + n)],
                            lhsT=corr_t[rg : rg + 64, 128 * cid : 128 * (cid + 1)],
                            rhs=irep_t[rg : rg + 64, 0 : 64 * n],
                            start=False,
                            stop=True,
                            skip_group_check=True,
                        )

                    lanes[b].append(_cr)
            for i2 in range(max(len(lanes[0]), len(lanes[1]))):
                for b in range(2):
                    if i2 < len(lanes[b]):
                        lanes[b][i2]()

            pt = pts.tile([128, 64 * CHUNK], F16, tag="pt")
            nc.scalar.activation(
                out=pt[:, : 64 * L],
                in_=ps[:, : 64 * L],
                func=mybir.ActivationFunctionType.Exp,
                bias=bias0[:],
                scale=0.125,
            )
            pend.append((pt, chunk, heads[s]))
            if len(pend) > DEPTH:
                emit_pv()
            # prefetch: issue head s+2's HBM loads after head s's first chunk
            if s + 2 == loaded + 0 and loaded < HPC:
                load_head(loaded)
                loaded += 1
        while pend:
            emit_pv()

    _split_multi_waits(nc)
    return nc


def _assignment(schedules):
    """Balanced head->core assignment: greedy longest-first onto the least
    loaded core with capacity HPC. Cost = column count (drives every stage).
    Returns list of N_CORES lists of flat head ids."""
    costs = [(len(schedules[g]["cols"]), g) for g in range(B * H)]
    costs.sort(reverse=True)
    loads = [0.0] * N_CORES
    slots = [[] for _ in range(N_CORES)]
    for cost, g in costs:
        c = min(
            (c for c in range(N_CORES) if len(slots[c]) < HPC),
            key=lambda c: (loads[c], c),
        )
        slots[c].append(g)
        loads[c] += cost
    return [sorted(sl) for sl in slots]


def _corr_patterns():
    """[64, N_CASES*128] fp16: stationary A with A.T @ I64 = U_case, where
    U_case [128, 64] holds the -2000 mask for (top, bot) case halves."""
    strict = np.tril(np.full((64, 64), NEG, np.float32), k=-1)  # kl > ql
    dead = np.full((64, 64), NEG, np.float32)
    live = np.zeros((64, 64), np.float32)
    half = {0: live, 1: dead, 2: strict}
    out = np.zeros((64, N_CASES * 128), np.float16)
    for (tc, bc), cid in CASE_IDS.items():
        U = np.concatenate([half[tc], half[bc]], axis=0)  # [128, 64]
        out[:, 128 * cid : 128 * (cid + 1)] = U.T.astype(np.float16)
    return np.ascontiguousarray(out)


def _prep_inputs(q, k, v, schedules):
    """Per-core input arrays keyed as the programs expect."""
    assign = _assignment(schedules)
    # flat head g = b*H + h
    qt_all = np.ascontiguousarray(
        q.transpose(0, 2, 3, 1).reshape(B * H, D, S).astype(np.float16)
    )
    kt_nat = k.transpose(0, 2, 3, 1).reshape(B * H, D, S).astype(np.float16)
    kt_nat = kt_nat.reshape(B * H, D, NB, 64)
    kt_all = np.empty_like(kt_nat)
    for g in range(B * H):
        order = [kb for p in schedules[g]["pairs"] for kb in p]
        kt_all[g] = kt_nat[g][:, order, :]
    kt_all = np.ascontiguousarray(kt_all.reshape(B * H, D, S))
    v_aug = np.concatenate([v, np.ones((B, S, H, 1), v.dtype)], axis=3)  # [B,S,H,65]
    v_aug = v_aug * VSCALE
    vb_all = v_aug.transpose(0, 2, 1, 3).reshape(B * H, NB, 64, 65)  # [g, kb, 64, 65]
    # va[g]: per pair t, rows 0:64 = V[kb1] block, rows 64:128 = V[kb2]
    va_all = np.zeros((B * H, 128, 16 * 65), np.float16)
    for g in range(B * H):
        for t, (kb1, kb2) in enumerate(schedules[g]["pairs"]):
            va_all[g, 0:64, 65 * t : 65 * (t + 1)] = vb_all[g, kb1]
            va_all[g, 64:128, 65 * t : 65 * (t + 1)] = vb_all[g, kb2]
    corr_half = _corr_patterns()
    corr_full = np.ascontiguousarray(np.concatenate([corr_half, corr_half], axis=0))
    irep_half = np.tile(np.eye(64, dtype=np.float16), (1, 8))
    irep_full = np.ascontiguousarray(np.concatenate([irep_half, irep_half], axis=0))
    in_maps = []
    for c in range(N_CORES):
        gs = assign[c]
        in_maps.append(
            {
                "qt": np.ascontiguousarray(qt_all[gs]),
                "kt": np.ascontiguousarray(kt_all[gs]),
                "va": np.ascontiguousarray(va_all[gs]),
                "corr": corr_full,
                "irep": irep_full,
            }
        )
    return in_maps


def _schedules(block_mask):
    """Per flat head: k-block pairing + column schedule."""
    masks_all = np.asarray(block_mask).reshape(B * H, NB, NB)
    scheds = []
    for g in range(B * H):
        if PAIRING == "match":
            pairs = _match_pairs(masks_all[g])
        else:
            pairs = [(2 * t, 2 * t + 1) for t in range(NB // 2)]
        scheds.append({"pairs": pairs, "cols": _head_schedule(masks_all[g], pairs)})
    return scheds


_PROG_CACHE = {}


def _get_programs(block_mask, schedules):
    key = np.asarray(block_mask).tobytes()
    if key not in _PROG_CACHE:
        assign = _assignment(schedules)
        _PROG_CACHE[key] = [
            build_program([schedules[g] for g in assign[c]]) for c in range(N_CORES)
        ]
    return _PROG_CACHE[key]


def run_cores(ncs, in_maps, trace=False):
    """Run the 8 per-core programs concurrently on the 8 devices."""
    import jax

    devs = jax.devices()
    results = [None] * N_CORES
    errs = [None] * N_CORES

    def _run(c):
        try:
            with jax.default_device(devs[c]):
                r = run_bass_kernel_spmd(
                    ncs[c], [in_maps[c]], core_ids=[0], trace=trace and c == 0
                )
                results[c] = r
        except Exception as e:  # noqa: BLE001
            errs[c] = e

    threads = [threading.Thread(target=_run, args=(c,)) for c in range(N_CORES)]
    for t in threads:
        t.start()
    for t in threads:
        t.join()
    for c, e in enumerate(errs):
        if e is not None:
            raise RuntimeError(f"core {c} failed") from e
    return results


def kernel(q, k, v, block_mask):
    q = np.asarray(q, dtype=np.float32)
    k = np.asarray(k, dtype=np.float32)
    v = np.asarray(v, dtype=np.float32)
    block_mask = np.asarray(block_mask).astype(bool)

    schedules = _schedules(block_mask)
    assign = _assignment(schedules)
    in_maps = _prep_inputs(q, k, v, schedules)
    ncs = _get_programs(block_mask, schedules)
    results = run_cores(ncs, in_maps)

    out = np.empty((B, S, H, D), np.float32)
    for c in range(N_CORES):
        ot = results[c].results[0]["ot"]  # [HPC, 65, S]
        for s in range(HPC):
            g = assign[c][s]
            b, h = divmod(g, H)
            o_un = ot[s, :D, :].astype(np.float32)  # [D, S] unnormalized (x1/16)
            l = ot[s, D, :].astype(np.float32)  # [S] denominator (x1/16)
            out[b, :, h, :] = (o_un / l[None, :]).T
    return out
